# revision 5
# baseline (speedup 1.0000x reference)
"""MiniDeepSeekV3Gate (noaux-topk MoE routing) Trainium2 Bass kernel.

Problem: T=16384 tokens, H=2048 hidden, E=256 experts, 8 groups of 32,
top-2-per-group sums -> top-4 groups -> top-8 experts -> normalized
sigmoid gate weights (scaled 2.5) + int32 expert indices.

Sharding: pure data parallel over tokens. Each of the 8 NeuronCores gets
2048 tokens and a replicated copy of the (256, 2048) gate weight + bias.
No cross-core communication.

Numeric scheme (split-precision matmul, ~1.1e-5 rms logit error, validated
on hardware vs fp32):
  logits*S = hi16.w_hi16 + [lo8.w_hi8 + hi8.w_lo8]   (S = 2048)
  hi16 = f16(S*x)            (ScalarE, from PE-transposed fp32 PSUM)
  lo8  = e4m3(S*x - hi16)    (VectorE scalar_tensor_tensor)
  hi8  = e4m3(hi16/S)        (GPSIMD tensor_scalar, SBUF source)
  w_hi16 = f16(w); w_hi8 = e4m3(w_hi16); w_lo8 = e4m3((w-w_hi16)*S)
The two fp8 correction terms run as ONE DoubleRow matmul per k-chunk
(contraction pairs = AP dim1 planes), i.e. 0.5 cyc/row. All three passes
accumulate into the same PSUM group; sigmoid applies 1/S via activation
scale. This cuts TensorE time ~4x vs the fp32 baseline (fp32 matmul is
4 cyc/row) while keeping top-k index flips negligible (~17/131072,
harness rel err ~9e-3 < 2e-2).

Dataflow per core: x loaded naturally (contiguous DMA); PE transposes
128x128 fp32 blocks (2 cyc/row); the three split passes double as the
PSUM evacuation (no separate evac pass). Scores are computed expert-major
(PSUM [128e, Tc]), sigmoid'd on ScalarE, PE-transposed back to
token-major, then the baseline-proven VectorE routing chain runs:
per-group Max8 -> top-2 sums -> top-4 group threshold mask ->
masked Max8/MaxIndex over 256 -> normalize.
"""

import numpy as np

import concourse.bass as bass
import concourse.tile as tile
from concourse import bacc, mybir
from concourse.bass_utils import run_bass_kernel_spmd
from concourse.masks import make_identity

F32 = mybir.dt.float32
F16 = mybir.dt.float16
F8 = mybir.dt.float8e4
I32 = mybir.dt.int32
U32 = mybir.dt.uint32
SIG = mybir.ActivationFunctionType.Sigmoid
COPY = mybir.ActivationFunctionType.Copy
ALU = mybir.AluOpType
DR = mybir.MatmulPerfMode.DoubleRow

N_CORES = 8
T_FULL = 16384
T_CORE = T_FULL // N_CORES  # 2048
HID = 2048
NE = 256
NG = 8
EPG = 32
TOPK = 8
ROUTE_SCALE = 2.5
NK = HID // 128  # 16 contraction chunks
S = 2048.0
BIG = 1.0e30

# token chunks: first small for a short prologue, last small for a short
# routing epilogue
CH = [128, 384, 384, 384, 384, 256, 128]


def build_nc(repeat=1):
    nc = bacc.Bacc("TRN2", target_bir_lowering=False, debug=False,
                   num_devices=N_CORES)
    x = nc.dram_tensor("hidden_states", [T_CORE, HID], F32,
                       kind="ExternalInput").ap()
    w = nc.dram_tensor("weight", [NE, HID], F32, kind="ExternalInput").ap()
    b = nc.dram_tensor("bias", [NE], F32, kind="ExternalInput").ap()
    out_w = nc.dram_tensor("weights_out", [T_CORE, TOPK], F32,
                           kind="ExternalOutput").ap()
    out_i = nc.dram_tensor("indices_out", [T_CORE, TOPK], I32,
                           kind="ExternalOutput").ap()

    with tile.TileContext(nc) as tc:
        for _ in range(repeat):
            build_tile_kernel(tc, x, w, b, out_w, out_i)
    nc.compile()
    return nc


def build_tile_kernel(tc, x, w, b, out_w, out_i):
    nc = tc.nc
    from contextlib import ExitStack
    ctx = ExitStack()
    with ctx:
        consts = ctx.enter_context(tc.tile_pool(name="consts", bufs=1))
        xn_pool = ctx.enter_context(tc.tile_pool(name="xn", bufs=4))
        xt_pool = ctx.enter_context(tc.tile_pool(name="xt", bufs=2))
        sg_pool = ctx.enter_context(tc.tile_pool(name="sg", bufs=2))
        st_pool = ctx.enter_context(tc.tile_pool(name="st", bufs=2))
        rt_pool = ctx.enter_context(tc.tile_pool(name="rt", bufs=2))
        outst_pool = ctx.enter_context(tc.tile_pool(name="outst", bufs=2))
        ps_x = ctx.enter_context(tc.tile_pool(name="ps_x", bufs=2,
                                              space="PSUM"))
        ps_s = ctx.enter_context(tc.tile_pool(name="ps_s", bufs=2,
                                              space="PSUM"))
        ps_t = ctx.enter_context(tc.tile_pool(name="ps_t", bufs=2,
                                              space="PSUM"))

        # ---- constants ----
        ident = consts.tile([128, 128], F32)
        make_identity(nc, ident[:])

        # ---- W prep: transpose + split ----
        # wt layout goals:
        #   w_hi16 [128k, NK, 2eb, 128e] fp16
        #   w_pack8 [128k, NK, 2pl, 2eb, 128e] fp8 (pl0=w_hi8, pl1=w_lo8s)
        wn = consts.tile([128, 2, HID], F32)
        for eb in range(2):
            nc.sync.dma_start(wn[:, eb, :], w[eb * 128:(eb + 1) * 128, :])
        w_hi16 = consts.tile([128, NK, 2, 128], F16)
        w_pack8 = consts.tile([128, NK, 2, 2, 128], F8)
        w_lo = consts.tile([128, NK, 2, 128], F32)
        for kg in range(NK // 4):
            pw = ps_x.tile([128, 4, 2, 128], F32, name=f"pw_{kg}",
                           tag="ps_x")
            for j in range(4):
                k = kg * 4 + j
                for eb in range(2):
                    nc.tensor.transpose(pw[:, j, eb, :],
                                        wn[:, eb, k * 128:(k + 1) * 128],
                                        ident[:])
            sl = slice(kg * 4, kg * 4 + 4)
            nc.scalar.activation(w_hi16[:, sl], pw[:], COPY)
            nc.vector.tensor_tensor(out=w_lo[:, sl], in0=pw[:],
                                    in1=w_hi16[:, sl], op=ALU.subtract)
            nc.gpsimd.tensor_scalar(out=w_pack8[:, sl, 0], in0=w_hi16[:, sl],
                                    scalar1=1.0, scalar2=None, op0=ALU.mult)
            nc.scalar.activation(w_pack8[:, sl, 1], w_lo[:, sl], COPY,
                                 scale=S)

        bias_bc = consts.tile([128, NE], F32)
        nc.sync.dma_start(bias_bc[:], b.unsqueeze(0).partition_broadcast(128))

        # ---- main loop: software-pipelined over token chunks ----
        OFF = [sum(CH[:i]) for i in range(len(CH))]
        NCH = len(CH)
        xts = {}
        pss = {}

        def emit_chunk_stage1(c):
            """DMA + transposes + split-evacs for chunk c."""
            ntt = CH[c] // 128
            t0 = OFF[c]
            xt16 = xt_pool.tile([128, NK, CH[c]], F16, name=f"xt16_{c}",
                                tag="xt16")
            xt8 = xt_pool.tile([128, NK, 2, CH[c]], F8, name=f"xt8_{c}",
                               tag="xt8")
            xts[c] = (xt16, xt8)
            for tt in range(ntt):
                xn = xn_pool.tile([128, HID], F32, name=f"xn_{c}_{tt}",
                                  tag="xn")
                nc.sync.dma_start(xn[:],
                                  x[t0 + tt * 128:t0 + (tt + 1) * 128, :])
                tsl = slice(tt * 128, (tt + 1) * 128)
                for kg in range(2):
                    px = ps_x.tile([128, 8, 128], F32,
                                   name=f"px_{c}_{tt}_{kg}", tag="ps_x")
                    for j in range(8):
                        k = kg * 8 + j
                        nc.tensor.transpose(px[:, j, :],
                                            xn[:, k * 128:(k + 1) * 128],
                                            ident[:])
                    ksl = slice(kg * 8, kg * 8 + 8)
                    nc.scalar.activation(xt16[:, ksl, tsl], px[:], COPY,
                                         scale=S)
                    nc.vector.scalar_tensor_tensor(
                        out=xt8[:, ksl, 0, tsl], in0=px[:], scalar=S,
                        in1=xt16[:, ksl, tsl], op0=ALU.mult,
                        op1=ALU.subtract)
                nc.gpsimd.tensor_scalar(out=xt8[:, :, 1, tsl],
                                        in0=xt16[:, :, tsl], scalar1=1.0 / S,
                                        scalar2=None, op0=ALU.mult)

        def emit_mm(c, mi):
            """One matmul unit for chunk c: mi in [0, 64)."""
            xt16, xt8 = xts[c]
            k, sub = divmod(mi, 4)
            eb, is_dr = divmod(sub, 2)
            if (c, eb) not in pss:
                pss[(c, eb)] = ps_s.tile([128, CH[c]], F32,
                                         name=f"ps_{c}_{eb}", tag="ps_s")
            if not is_dr:
                nc.tensor.matmul(pss[(c, eb)][:], w_hi16[:, k, eb, :],
                                 xt16[:, k, :], start=(k == 0), stop=False)
            else:
                nc.tensor.matmul(pss[(c, eb)][:], w_pack8[:, k, :, eb, :],
                                 xt8[:, k, :, :], start=False,
                                 stop=(k == NK - 1), perf_mode=DR)

        def emit_tail(c):
            """Sigmoid + transpose-back + routing + out DMA for chunk c."""
            ntt = CH[c] // 128
            t0 = OFF[c]
            sgs = []
            for eb in range(2):
                sg = sg_pool.tile([128, CH[c]], F32, name=f"sg_{c}_{eb}",
                                  tag="sg")
                nc.scalar.activation(sg[:], pss.pop((c, eb))[:], SIG,
                                     scale=1.0 / S)
                sgs.append(sg)

            wo = outst_pool.tile([128, ntt, TOPK], F32, name=f"wo_{c}",
                                 tag="wo")
            io = outst_pool.tile([128, ntt, TOPK], U32, name=f"io_{c}",
                                 tag="io")

            for tt in range(ntt):
                pt = ps_t.tile([128, NE], F32, name=f"pt_{c}_{tt}",
                               tag="ps_t")
                for eb in range(2):
                    nc.tensor.transpose(pt[:, eb * 128:(eb + 1) * 128],
                                        sgs[eb][:, tt * 128:(tt + 1) * 128],
                                        ident[:])
                # token-major selection scores = sigmoid + bias
                st = st_pool.tile([128, NE], F32, name=f"st_{c}_{tt}",
                                  tag="st")
                nc.vector.tensor_tensor(out=st[:], in0=pt[:], in1=bias_bc[:],
                                        op=ALU.add)

                gtop = rt_pool.tile([128, NG, 8], F32, name=f"gt_{c}_{tt}",
                                    tag="gt")
                for g in range(NG):
                    nc.vector.max(gtop[:, g, :], st[:, g * EPG:(g + 1) * EPG])
                g2 = rt_pool.tile([128, NG], F32, name=f"g2_{c}_{tt}",
                                  tag="g2")
                nc.vector.tensor_tensor(out=g2[:], in0=gtop[:, :, 0],
                                        in1=gtop[:, :, 1], op=ALU.add)
                gs8 = rt_pool.tile([128, NG], F32, name=f"gs8_{c}_{tt}",
                                   tag="gs8")
                nc.vector.max(gs8[:], g2[:])
                maskg = rt_pool.tile([128, NG], F32, name=f"mg_{c}_{tt}",
                                     tag="mg")
                nc.vector.tensor_scalar(out=maskg[:], in0=g2[:],
                                        scalar1=gs8[:, 3:4], scalar2=BIG,
                                        op0=ALU.is_ge, op1=ALU.mult)
                masked = rt_pool.tile([128, NE], F32, name=f"mk_{c}_{tt}",
                                      tag="mk")
                nc.vector.scalar_tensor_tensor(
                    out=masked[:].rearrange("p (g e) -> p g e", g=NG),
                    in0=maskg[:].unsqueeze(2).broadcast_to((128, NG, EPG)),
                    scalar=BIG,
                    in1=st[:].rearrange("p (g e) -> p g e", g=NG),
                    op0=ALU.subtract, op1=ALU.add)
                top8v = rt_pool.tile([128, TOPK], F32, name=f"t8_{c}_{tt}",
                                     tag="t8")
                nc.vector.max(top8v[:], masked[:])
                nc.vector.max_index(io[:, tt, :], top8v[:], masked[:])
                ssum = rt_pool.tile([128, 1], F32, name=f"ss_{c}_{tt}",
                                    tag="ss")
                nc.vector.reduce_sum(out=ssum[:], in_=top8v[:],
                                     axis=mybir.AxisListType.X)
                seps = rt_pool.tile([128, 1], F32, name=f"se_{c}_{tt}",
                                    tag="se")
                nc.vector.tensor_scalar_add(seps[:], ssum[:], 1e-6)
                rinv = rt_pool.tile([128, 1], F32, name=f"ri_{c}_{tt}",
                                    tag="ri")
                nc.vector.reciprocal(rinv[:], seps[:])
                nc.vector.tensor_scalar(out=wo[:, tt, :], in0=top8v[:],
                                        scalar1=rinv[:], scalar2=ROUTE_SCALE,
                                        op0=ALU.mult, op1=ALU.mult)

            nc.sync.dma_start(
                out_w[t0:t0 + CH[c], :].rearrange("(tt p) k -> p tt k",
                                                  tt=ntt),
                wo[:])
            nc.sync.dma_start(
                out_i[t0:t0 + CH[c], :].rearrange("(tt p) k -> p tt k",
                                                  tt=ntt),
                io[:].bitcast(I32))

        # pipeline: stage1(c) emits while mm(c-1) interleaves; tail(c-1)
        # after. Interleave mm units between stage1's per-tt groups.
        for c in range(NCH + 1):
            if c < NCH:
                emit_chunk_stage1(c)
            if c >= 1:
                for mi in range(64):
                    emit_mm(c - 1, mi)
                emit_tail(c - 1)


_NC_CACHE = None


def _get_nc():
    global _NC_CACHE
    if _NC_CACHE is None:
        _NC_CACHE = build_nc()
    return _NC_CACHE


def kernel(hidden_states: np.ndarray, weight: np.ndarray, bias: np.ndarray):
    hidden_states = np.ascontiguousarray(hidden_states, dtype=np.float32)
    weight = np.ascontiguousarray(weight, dtype=np.float32)
    bias = np.ascontiguousarray(bias, dtype=np.float32)
    nc = _get_nc()
    in_maps = [
        {
            "hidden_states": hidden_states[c * T_CORE:(c + 1) * T_CORE],
            "weight": weight,
            "bias": bias,
        }
        for c in range(N_CORES)
    ]
    res = run_bass_kernel_spmd(nc, in_maps, list(range(N_CORES))).results
    weights = np.concatenate([r["weights_out"] for r in res], axis=0)
    indices = np.concatenate([r["indices_out"] for r in res], axis=0)
    return weights.astype(np.float32), indices.astype(np.int32)


# revision 8
# speedup vs baseline: 1.0382x; 1.0382x over previous
"""MiniDeepSeekV3Gate (noaux-topk MoE routing) Trainium2 Bass kernel.

Problem: T=16384 tokens, H=2048 hidden, E=256 experts, 8 groups of 32,
top-2-per-group sums -> top-4 groups -> top-8 experts -> normalized
sigmoid gate weights (scaled 2.5) + int32 expert indices.

Sharding: pure data parallel over tokens. Each of the 8 NeuronCores gets
2048 tokens and a replicated copy of the (256, 2048) gate weight + bias.
No cross-core communication.

Numeric scheme (split-precision matmul, ~1.1e-5 rms logit error, validated
on hardware vs fp32):
  logits*S = hi16.w_hi16 + [lo8.w_hi8 + hi8.w_lo8]   (S = 2048)
  hi16 = f16(S*x)            (ScalarE, from PE-transposed fp32 PSUM)
  lo8  = e4m3(S*x - hi16)    (VectorE scalar_tensor_tensor)
  hi8  = e4m3(hi16/S)        (GPSIMD tensor_scalar, SBUF source)
  w_hi16 = f16(w); w_hi8 = e4m3(w_hi16); w_lo8 = e4m3((w-w_hi16)*S)
The two fp8 correction terms run as ONE DoubleRow matmul per k-chunk
(contraction pairs = AP dim1 planes), i.e. 0.5 cyc/row. All three passes
accumulate into the same PSUM group; sigmoid applies 1/S via activation
scale. This cuts TensorE time ~4x vs the fp32 baseline (fp32 matmul is
4 cyc/row) while keeping top-k index flips negligible (~17/131072,
harness rel err ~9e-3 < 2e-2).

Dataflow per core: x loaded naturally (contiguous DMA); PE transposes
128x128 fp32 blocks (2 cyc/row); the three split passes double as the
PSUM evacuation (no separate evac pass). Scores are computed expert-major
(PSUM [128e, Tc]), sigmoid'd on ScalarE, PE-transposed back to
token-major, then the baseline-proven VectorE routing chain runs:
per-group Max8 -> top-2 sums -> top-4 group threshold mask ->
masked Max8/MaxIndex over 256 -> normalize.
"""

import numpy as np

import concourse.bass as bass
import concourse.tile as tile
from concourse import bacc, mybir
from concourse.bass_utils import run_bass_kernel_spmd
from concourse.masks import make_identity

F32 = mybir.dt.float32
F16 = mybir.dt.float16
F8 = mybir.dt.float8e4
I32 = mybir.dt.int32
U32 = mybir.dt.uint32
SIG = mybir.ActivationFunctionType.Sigmoid
COPY = mybir.ActivationFunctionType.Copy
ALU = mybir.AluOpType
DR = mybir.MatmulPerfMode.DoubleRow

N_CORES = 8
T_FULL = 16384
T_CORE = T_FULL // N_CORES  # 2048
HID = 2048
NE = 256
NG = 8
EPG = 32
TOPK = 8
ROUTE_SCALE = 2.5
NK = HID // 128  # 16 contraction chunks
S = 2048.0
BIG = 1.0e30

# token chunks: first small for a short prologue, last small for a short
# routing epilogue
CH = [128, 384, 384, 384, 384, 256, 128]


def build_nc(repeat=1):
    nc = bacc.Bacc("TRN2", target_bir_lowering=False, debug=False,
                   num_devices=N_CORES)
    x = nc.dram_tensor("hidden_states", [T_CORE, HID], F32,
                       kind="ExternalInput").ap()
    w = nc.dram_tensor("weight", [NE, HID], F32, kind="ExternalInput").ap()
    b = nc.dram_tensor("bias", [NE], F32, kind="ExternalInput").ap()
    out_w = nc.dram_tensor("weights_out", [T_CORE, TOPK], F32,
                           kind="ExternalOutput").ap()
    out_i = nc.dram_tensor("indices_out", [T_CORE, TOPK], I32,
                           kind="ExternalOutput").ap()

    with tile.TileContext(nc) as tc:
        for _ in range(repeat):
            build_tile_kernel(tc, x, w, b, out_w, out_i)
    nc.compile()
    return nc


def build_tile_kernel(tc, x, w, b, out_w, out_i):
    nc = tc.nc
    from contextlib import ExitStack
    ctx = ExitStack()
    with ctx:
        consts = ctx.enter_context(tc.tile_pool(name="consts", bufs=1))
        xn_pool = ctx.enter_context(tc.tile_pool(name="xn", bufs=4))
        xt_pool = ctx.enter_context(tc.tile_pool(name="xt", bufs=2))
        sg_pool = ctx.enter_context(tc.tile_pool(name="sg", bufs=2))
        st_pool = ctx.enter_context(tc.tile_pool(name="st", bufs=2))
        rt_pool = ctx.enter_context(tc.tile_pool(name="rt", bufs=2))
        outst_pool = ctx.enter_context(tc.tile_pool(name="outst", bufs=2))
        ps_x = ctx.enter_context(tc.tile_pool(name="ps_x", bufs=2,
                                              space="PSUM"))
        ps_s = ctx.enter_context(tc.tile_pool(name="ps_s", bufs=2,
                                              space="PSUM"))
        ps_t = ctx.enter_context(tc.tile_pool(name="ps_t", bufs=2,
                                              space="PSUM"))

        # ---- constants ----
        ident = consts.tile([128, 128], F32)
        make_identity(nc, ident[:])

        # ---- W prep: transpose + split ----
        # wt layout goals:
        #   w_hi16 [128k, NK, 2eb, 128e] fp16
        #   w_pack8 [128k, NK, 2pl, 2eb, 128e] fp8 (pl0=w_hi8, pl1=w_lo8s)
        # W prep state (DMA issued after the first x chunk so the PE can
        # start on x transposes as early as possible)
        wn = consts.tile([128, 2, HID], F32)
        w_hi16 = consts.tile([128, NK, 2, 128], F16)
        w_pack8 = consts.tile([128, NK, 2, 2, 128], F8)
        w_lo = consts.tile([128, NK, 2, 128], F32)
        bias_bc = consts.tile([128, NE], F32)

        def emit_w_prep():
            for eb in range(2):
                nc.sync.dma_start(wn[:, eb, :], w[eb * 128:(eb + 1) * 128, :])
            for kg in range(NK // 4):
                pw = ps_x.tile([128, 4, 2, 128], F32, name=f"pw_{kg}",
                               tag="ps_x")
                for j in range(4):
                    k = kg * 4 + j
                    for eb in range(2):
                        nc.tensor.transpose(pw[:, j, eb, :],
                                            wn[:, eb, k * 128:(k + 1) * 128],
                                            ident[:])
                sl = slice(kg * 4, kg * 4 + 4)
                nc.scalar.activation(w_hi16[:, sl], pw[:], COPY)
                nc.vector.tensor_tensor(out=w_lo[:, sl], in0=pw[:],
                                        in1=w_hi16[:, sl], op=ALU.subtract)
                nc.gpsimd.tensor_scalar(out=w_pack8[:, sl, 0],
                                        in0=w_hi16[:, sl], scalar1=1.0,
                                        scalar2=None, op0=ALU.mult)
                nc.scalar.activation(w_pack8[:, sl, 1], w_lo[:, sl], COPY,
                                     scale=S)
            nc.sync.dma_start(bias_bc[:],
                              b.unsqueeze(0).partition_broadcast(128))

        # ---- main loop: software-pipelined over token chunks ----
        OFF = [sum(CH[:i]) for i in range(len(CH))]
        NCH = len(CH)
        xts = {}
        pss = {}

        def emit_stage1_tt(c, tt):
            """DMA + transposes + split-evacs for 128 tokens of chunk c."""
            ntt = CH[c] // 128
            t0 = OFF[c]
            if tt == 0:
                xt16 = xt_pool.tile([128, NK, CH[c]], F16, name=f"xt16_{c}",
                                    tag="xt16")
                xt8 = xt_pool.tile([128, NK, 2, CH[c]], F8, name=f"xt8_{c}",
                                   tag="xt8")
                xts[c] = (xt16, xt8)
            xt16, xt8 = xts[c]
            xn = xn_pool.tile([128, HID], F32, name=f"xn_{c}_{tt}",
                              tag="xn")
            nc.sync.dma_start(xn[:],
                              x[t0 + tt * 128:t0 + (tt + 1) * 128, :])
            tsl = slice(tt * 128, (tt + 1) * 128)
            for kg in range(2):
                px = ps_x.tile([128, 8, 128], F32,
                               name=f"px_{c}_{tt}_{kg}", tag="ps_x")
                for j in range(8):
                    k = kg * 8 + j
                    nc.tensor.transpose(px[:, j, :],
                                        xn[:, k * 128:(k + 1) * 128],
                                        ident[:])
                ksl = slice(kg * 8, kg * 8 + 8)
                nc.scalar.activation(xt16[:, ksl, tsl], px[:], COPY,
                                     scale=S)
                nc.vector.scalar_tensor_tensor(
                    out=xt8[:, ksl, 0, tsl], in0=px[:], scalar=S,
                    in1=xt16[:, ksl, tsl], op0=ALU.mult,
                    op1=ALU.subtract)
            nc.gpsimd.tensor_scalar(out=xt8[:, :, 1, tsl],
                                    in0=xt16[:, :, tsl], scalar1=1.0 / S,
                                    scalar2=None, op0=ALU.mult)

        def emit_mm(c, mi):
            """One matmul unit for chunk c: mi in [0, 64).

            All 32 fp16 P1 units come first (they only need xt16, which the
            ScalarE produces first), then the 32 fp8 DoubleRow units (which
            need the VectorE/GPSIMD planes of the whole chunk).
            """
            xt16, xt8 = xts[c]
            if mi < 32:
                k, eb = divmod(mi, 2)
                if (c, eb) not in pss:
                    pss[(c, eb)] = ps_s.tile([128, CH[c]], F32,
                                             name=f"ps_{c}_{eb}", tag="ps_s")
                nc.tensor.matmul(pss[(c, eb)][:], w_hi16[:, k, eb, :],
                                 xt16[:, k, :], start=(k == 0), stop=False)
            else:
                k, eb = divmod(mi - 32, 2)
                nc.tensor.matmul(pss[(c, eb)][:], w_pack8[:, k, :, eb, :],
                                 xt8[:, k, :, :], start=False,
                                 stop=(k == NK - 1), perf_mode=DR)

        def emit_tail(c):
            """Sigmoid + transpose-back + routing + out DMA for chunk c."""
            ntt = CH[c] // 128
            t0 = OFF[c]
            sgs = []
            for eb in range(2):
                sg = sg_pool.tile([128, CH[c]], F32, name=f"sg_{c}_{eb}",
                                  tag="sg")
                nc.scalar.activation(sg[:], pss.pop((c, eb))[:], SIG,
                                     scale=1.0 / S)
                sgs.append(sg)

            wo = outst_pool.tile([128, ntt, TOPK], F32, name=f"wo_{c}",
                                 tag="wo")
            io = outst_pool.tile([128, ntt, TOPK], U32, name=f"io_{c}",
                                 tag="io")

            for tt in range(ntt):
                pt = ps_t.tile([128, NE], F32, name=f"pt_{c}_{tt}",
                               tag="ps_t")
                for eb in range(2):
                    nc.tensor.transpose(pt[:, eb * 128:(eb + 1) * 128],
                                        sgs[eb][:, tt * 128:(tt + 1) * 128],
                                        ident[:])
                # token-major selection scores = sigmoid + bias
                st = st_pool.tile([128, NE], F32, name=f"st_{c}_{tt}",
                                  tag="st")
                nc.vector.tensor_tensor(out=st[:], in0=pt[:], in1=bias_bc[:],
                                        op=ALU.add)

                gtop = rt_pool.tile([128, NG, 8], F32, name=f"gt_{c}_{tt}",
                                    tag="gt")
                for g in range(NG):
                    nc.vector.max(gtop[:, g, :], st[:, g * EPG:(g + 1) * EPG])
                g2 = rt_pool.tile([128, NG], F32, name=f"g2_{c}_{tt}",
                                  tag="g2")
                nc.vector.tensor_tensor(out=g2[:], in0=gtop[:, :, 0],
                                        in1=gtop[:, :, 1], op=ALU.add)
                gs8 = rt_pool.tile([128, NG], F32, name=f"gs8_{c}_{tt}",
                                   tag="gs8")
                nc.vector.max(gs8[:], g2[:])
                maskg = rt_pool.tile([128, NG], F32, name=f"mg_{c}_{tt}",
                                     tag="mg")
                nc.vector.tensor_scalar(out=maskg[:], in0=g2[:],
                                        scalar1=gs8[:, 3:4], scalar2=BIG,
                                        op0=ALU.is_ge, op1=ALU.mult)
                masked = rt_pool.tile([128, NE], F32, name=f"mk_{c}_{tt}",
                                      tag="mk")
                nc.vector.scalar_tensor_tensor(
                    out=masked[:].rearrange("p (g e) -> p g e", g=NG),
                    in0=maskg[:].unsqueeze(2).broadcast_to((128, NG, EPG)),
                    scalar=BIG,
                    in1=st[:].rearrange("p (g e) -> p g e", g=NG),
                    op0=ALU.subtract, op1=ALU.add)
                top8v = rt_pool.tile([128, TOPK], F32, name=f"t8_{c}_{tt}",
                                     tag="t8")
                nc.vector.max(top8v[:], masked[:])
                nc.vector.max_index(io[:, tt, :], top8v[:], masked[:])
                ssum = rt_pool.tile([128, 1], F32, name=f"ss_{c}_{tt}",
                                    tag="ss")
                nc.vector.reduce_sum(out=ssum[:], in_=top8v[:],
                                     axis=mybir.AxisListType.X)
                seps = rt_pool.tile([128, 1], F32, name=f"se_{c}_{tt}",
                                    tag="se")
                nc.vector.tensor_scalar_add(seps[:], ssum[:], 1e-6)
                rinv = rt_pool.tile([128, 1], F32, name=f"ri_{c}_{tt}",
                                    tag="ri")
                nc.vector.reciprocal(rinv[:], seps[:])
                nc.vector.tensor_scalar(out=wo[:, tt, :], in0=top8v[:],
                                        scalar1=rinv[:], scalar2=ROUTE_SCALE,
                                        op0=ALU.mult, op1=ALU.mult)

            nc.sync.dma_start(
                out_w[t0:t0 + CH[c], :].rearrange("(tt p) k -> p tt k",
                                                  tt=ntt),
                wo[:])
            nc.sync.dma_start(
                out_i[t0:t0 + CH[c], :].rearrange("(tt p) k -> p tt k",
                                                  tt=ntt),
                io[:].bitcast(I32))

        # pipeline: chunk c's per-tt stage1 groups interleaved with chunk
        # c-1's matmul units so the PE stream stays dense and the evac
        # engines are fed continuously; W prep goes right after the first
        # tt group (its DMA then follows chunk 0's first x tile).
        for c in range(NCH + 1):
            ntt = CH[c] // 128 if c < NCH else 1
            mi = 0
            for tt in range(ntt):
                if c < NCH:
                    emit_stage1_tt(c, tt)
                if c == 0 and tt == 0:
                    emit_w_prep()
                if c >= 1:
                    want = (tt + 1) * 64 // ntt
                    while mi < want:
                        emit_mm(c - 1, mi)
                        mi += 1
            if c >= 1:
                emit_tail(c - 1)


_NC_CACHE = None


def _get_nc():
    global _NC_CACHE
    if _NC_CACHE is None:
        _NC_CACHE = build_nc()
    return _NC_CACHE


def kernel(hidden_states: np.ndarray, weight: np.ndarray, bias: np.ndarray):
    hidden_states = np.ascontiguousarray(hidden_states, dtype=np.float32)
    weight = np.ascontiguousarray(weight, dtype=np.float32)
    bias = np.ascontiguousarray(bias, dtype=np.float32)
    nc = _get_nc()
    in_maps = [
        {
            "hidden_states": hidden_states[c * T_CORE:(c + 1) * T_CORE],
            "weight": weight,
            "bias": bias,
        }
        for c in range(N_CORES)
    ]
    res = run_bass_kernel_spmd(nc, in_maps, list(range(N_CORES))).results
    weights = np.concatenate([r["weights_out"] for r in res], axis=0)
    indices = np.concatenate([r["indices_out"] for r in res], axis=0)
    return weights.astype(np.float32), indices.astype(np.int32)
